# revision 1
# baseline (speedup 1.0000x reference)
"""Trainium2 Bass kernel for nn_DSVDD (retrieval_knn).

Math (per batch b):
  phi = W @ p_b + bias            [DIM, HW]    (1x1 conv)
  sqdist[i,j] = ||phi_i||^2 + ||C_j||^2 - 2 phi_i . C_j
  top-3 smallest distances d0<=d1<=d2  ->  w0 = 1/(1+exp(d0-d1)+exp(d0-d2))
  score[i] = w0 * d0

Device strategy (8 cores, data-parallel over (batch, HW-half)):
  Y[i,j] = 2 phi_i . C_j - ||C_j||^2.  The 2C part runs as fp32r PE matmuls;
  the -c_j correction is materialized once per j-slice ([128, js] via a
  ones-matmul on the replicated -c/128 block) and applied by a DVE add.
  top-3 smallest sqdist == top-3 largest Y (f_i = ||phi_i||^2 common per row).
  DVE max8 finds the top-8 largest Y per row in one instruction; streamed
  merge over j-slices.  f_i via ones-matmuls over Squared phi (deferred one
  conv step so they never stall the PE).  Tail (sqrt, softmin) on ACT/DVE.
"""
import sys

sys.path.insert(0, "/opt/trn_rl_repo")

import numpy as np

B, DIM, H, W_ = 4, 1792, 56, 56
HW = H * W_            # 3136
P = 3136               # prototypes
NCORES = 8
HALF = HW // 2         # 1568 positions per core
KC = DIM // 128        # 14 contraction chunks
KCH = KC // 2          # 7 (p tiles split in halves for early start)
KCC = KC + 1           # 15 chunks in cb (incl. replicated -c/128 block)
IB = 392               # conv i-block (moving cols)
NIB = HALF // IB       # 4
JSLICES = [256, 480, 480, 480, 480, 480, 480]   # G-phase j-slices (sum 3136)
NIT = 13               # i-tiles: 12 full + 1 ragged(32)
LAST_W = HALF - 12 * 128   # 32

_cache = {}


def _build_program():
    import concourse.tile as tile
    from concourse import bacc, mybir

    F32 = mybir.dt.float32
    F32R = mybir.dt.float32r
    AF = mybir.ActivationFunctionType
    ALU = mybir.AluOpType
    AX = mybir.AxisListType

    nc = bacc.Bacc("TRN2", target_bir_lowering=False, debug=False)

    pT_d = nc.dram_tensor("pT", [DIM, HALF], F32R, kind="ExternalInput")
    wt_d = nc.dram_tensor("wt", [DIM, DIM], F32R, kind="ExternalInput")   # W^T
    cb_d = nc.dram_tensor("cb", [KCC * 128, P], F32R, kind="ExternalInput")
    bias_d = nc.dram_tensor("bias", [DIM], F32, kind="ExternalInput")
    onec_d = nc.dram_tensor("onec", [128, 1], F32R, kind="ExternalInput")
    oner_d = nc.dram_tensor("oner", [1, 128], F32R, kind="ExternalInput")
    ones2_d = nc.dram_tensor("ones2", [128, 128], F32R, kind="ExternalInput")
    score_d = nc.dram_tensor("score", [128, NIT], F32, kind="ExternalOutput")

    with tile.TileContext(nc) as tc:
        with (
            tc.tile_pool(name="persist", bufs=1) as persist,
            tc.tile_pool(name="cbp0", bufs=1) as cbp0,
        ):
            phi = persist.tile([128, KC, HALF], F32R)
            bias_col = persist.tile([128, KC], F32)
            onec = persist.tile([128, 1], F32R)
            oner = persist.tile([1, 128], F32R)
            ones2 = persist.tile([128, 128], F32R)
            f_row = persist.tile([1, HALF], F32)
            f_col = persist.tile([128, NIT], F32)
            runA = persist.tile([128, NIT, 8], F32)
            score_col = persist.tile([128, NIT], F32)

            # ------------- conv phase: phi = W @ p + b, f = ||phi||^2 -------
            with (
                tc.tile_pool(name="pp", bufs=6) as pp,
                tc.tile_pool(name="wtp", bufs=3) as wtp,
                tc.tile_pool(name="sqp", bufs=4) as sqp,
                tc.tile_pool(name="cps", bufs=4, space="PSUM") as cps,
                tc.tile_pool(name="fps", bufs=1, space="PSUM") as fps,
            ):
                f_ps = [fps.tile([1, IB], F32, name=f"fp{ib}", tag=f"f{ib}")
                        for ib in range(NIB)]

                def load_wt(dcg):
                    t = wtp.tile([128, KC, 128], F32R, name="wt_t")
                    nc.sync.dma_start(
                        t[:],
                        wt_d[:, dcg * 128:(dcg + 1) * 128].rearrange(
                            "(cc p) d -> p cc d", p=128),
                    )
                    return t

                def load_phalf(ib, h):
                    t = pp.tile([128, KCH, IB], F32R, name=f"pq{ib}{h}",
                                tag="pq")
                    nc.sync.dma_start(
                        t[:],
                        pT_d[h * KCH * 128:(h + 1) * KCH * 128,
                             ib * IB:(ib + 1) * IB].rearrange(
                            "(cc p) i -> p cc i", p=128),
                    )
                    return t

                # startup-critical loads first: wt chunk 0, then p halves
                dcg_seq = list(range(KC)) + list(reversed(range(KC)))  # snake
                wt_tiles = {0: load_wt(dcg_seq[0])}
                wt_issued = 1

                def wt_prefetch(upto):
                    nonlocal wt_issued
                    while wt_issued < min(upto, 2 * KC):
                        if dcg_seq[wt_issued] == dcg_seq[wt_issued - 1]:
                            # snake turn: same chunk again, reuse the tile
                            wt_tiles[wt_issued] = wt_tiles[wt_issued - 1]
                        else:
                            wt_tiles[wt_issued] = load_wt(dcg_seq[wt_issued])
                        wt_issued += 1

                # PE warmup: dummy matmuls keep HAM's activity monitor hot
                # while the first real DMAs land, so conv starts at 2.4 GHz.
                warm = pp.tile([128, 512], F32R, name="warm", tag="warm", bufs=1)
                nc.vector.memset(warm[:].bitcast(F32), 1.0)
                wps = cps.tile([128, 512], F32, name="wps", tag="acc")
                for _ in range(68):
                    nc.tensor.matmul(wps[:], warm[:, 0:128], warm[:],
                                     start=True, stop=True)

                cb0_t = None
                small_dmas_done = False
                pending_f = []
                for sub in range(2):
                    p_t = {}
                    for ib in (2 * sub, 2 * sub + 1):
                        p_t[ib] = [load_phalf(ib, 0), load_phalf(ib, 1)]
                    if not small_dmas_done:
                        small_dmas_done = True
                        nc.sync.dma_start(
                            bias_col[:],
                            bias_d.rearrange("(g p) -> p g", p=128))
                        nc.sync.dma_start(onec[:], onec_d[:])
                        nc.sync.dma_start(oner[:], oner_d[:])
                        nc.sync.dma_start(ones2[:], ones2_d[:])
                    for dcg_i in range(KC):
                        pos = sub * KC + dcg_i
                        dcg = dcg_seq[pos]
                        wt_t = wt_tiles.pop(pos)
                        wt_prefetch(pos + 3)
                        for k, ib in enumerate((2 * sub, 2 * sub + 1)):
                            if k == 1 and pending_f:
                                # deferred f matmuls: deps long satisfied
                                for args, kw in pending_f:
                                    nc.tensor.matmul(*args, **kw)
                                pending_f = []
                            acc = cps.tile([128, IB], F32)
                            for cc in range(KC):
                                nc.tensor.matmul(
                                    acc[:],
                                    wt_t[:, cc, :],
                                    p_t[ib][cc // KCH][:, cc % KCH, :],
                                    start=(cc == 0),
                                    stop=(cc == KC - 1),
                                )
                            isl = slice(ib * IB, (ib + 1) * IB)
                            # phi = psum + bias (rounded to fp32r)
                            nc.scalar.activation(
                                phi[:, dcg, isl], acc[:], AF.Identity,
                                bias=bias_col[:, dcg:dcg + 1],
                            )
                            # phi2 = (psum + bias)^2
                            sq = sqp.tile([128, IB], F32R)
                            nc.scalar.activation(
                                sq[:], acc[:], AF.Square,
                                bias=bias_col[:, dcg:dcg + 1],
                            )
                            pending_f.append((
                                (f_ps[ib][:], onec[:], sq[:]),
                                dict(start=(dcg_i == 0), stop=(dcg_i == KC - 1)),
                            ))
                    if sub == 0:
                        # prefetch first G slice mid-conv
                        j0 = JSLICES[0]
                        cb0_t = cbp0.tile([128, KCC, j0], F32R)
                        nc.sync.dma_start(
                            cb0_t[:],
                            cb_d[:, 0:j0].rearrange("(cc p) j -> p cc j",
                                                    p=128),
                        )
                for args, kw in pending_f:
                    nc.tensor.matmul(*args, **kw)
                pending_f = []
                for ib in range(NIB):
                    nc.vector.tensor_copy(
                        f_row[:, ib * IB:(ib + 1) * IB], f_ps[ib][:]
                    )

            # ------------- f relayout: [1, 1568] -> [128, 13] ---------------
            with tc.tile_pool(name="ftp", bufs=2, space="PSUM") as ftp:
                ft = ftp.tile([128, NIT], F32)
                for it in range(NIT):
                    w = 128 if it < 12 else LAST_W
                    nc.tensor.transpose(
                        ft[0:w, it:it + 1],
                        f_row[:, it * 128:it * 128 + w],
                        oner[0:1, 0:1].bitcast(F32),
                    )
                nc.scalar.activation(f_col[:], ft[:], AF.Copy)

            # ------------- G phase: Y = 2 phi.C - c, streamed top-8 ---------
            with (
                tc.tile_pool(name="cbp", bufs=2) as cbp,
                tc.tile_pool(name="cbcp", bufs=2) as cbcp,
                tc.tile_pool(name="ysb", bufs=4) as ysb,
                tc.tile_pool(name="mrg", bufs=4) as mrg,
                tc.tile_pool(name="yps", bufs=8, space="PSUM") as yps,
            ):
                joff = [0]
                for js in range(1, len(JSLICES)):
                    joff.append(joff[-1] + JSLICES[js - 1])

                for js in range(len(JSLICES)):
                    w_js = JSLICES[js]
                    jsl = slice(joff[js], joff[js] + w_js)
                    if js == 0:
                        cb_t = cb0_t
                    else:
                        cb_t = cbp.tile([128, KCC, w_js], F32R, name="cb_t",
                                        tag="cb")
                        nc.sync.dma_start(
                            cb_t[:],
                            cb_d[:, jsl].rearrange("(cc p) j -> p cc j",
                                                   p=128),
                        )
                    # materialize -c for this slice: ones2 @ (-c/128 block)
                    cps_t = yps.tile([128, 512], F32, name="y", tag="y")
                    nc.tensor.matmul(cps_t[:, 0:w_js], ones2[:],
                                     cb_t[:, KC, :], start=True, stop=True)
                    cbc_t = cbcp.tile([128, 512], F32, name="cbc_t")
                    nc.scalar.activation(cbc_t[:, 0:w_js], cps_t[:, 0:w_js],
                                         AF.Copy)
                    for it in range(NIT):
                        w = 128 if it < 12 else LAST_W
                        i0 = it * 128
                        y = yps.tile([128, 512], F32, name="y", tag="y")
                        for cc in range(KC):
                            nc.tensor.matmul(
                                y[0:w, 0:w_js],
                                phi[:, cc, i0:i0 + w],
                                cb_t[:, cc, :],
                                start=(cc == 0),
                                stop=(cc == KC - 1),
                            )
                        ys = ysb.tile([128, 512], F32, name="ys", tag="ys")
                        nc.vector.tensor_tensor(
                            ys[0:w, 0:w_js], y[0:w, 0:w_js],
                            cbc_t[0:w, 0:w_js], ALU.add,
                        )
                        if js == 0:
                            nc.vector.max(runA[0:w, it, :], ys[0:w, 0:w_js])
                        else:
                            m = mrg.tile([128, 16], F32)
                            nc.vector.tensor_copy(m[0:w, 0:8], runA[0:w, it, :])
                            nc.vector.max(m[0:w, 8:16], ys[0:w, 0:w_js])
                            nc.vector.max(runA[0:w, it, :], m[0:w, :])

                # ------------- tail: sqrt + softmin weight -------------------
                with tc.tile_pool(name="tails", bufs=4) as tails:
                    for it in range(NIT):
                        w = 128 if it < 12 else LAST_W
                        d3 = tails.tile([128, 3], F32, tag="d3")
                        nc.scalar.activation(
                            d3[0:w, :], runA[0:w, it, 0:3], AF.Sqrt,
                            bias=f_col[0:w, it:it + 1], scale=-1.0,
                        )
                        dd = tails.tile([128, 3], F32, tag="dd")
                        nc.vector.tensor_scalar(
                            dd[0:w, :], d3[0:w, :], d3[0:w, 0:1], None,
                            ALU.subtract,
                        )
                        ee = tails.tile([128, 3], F32, tag="ee")
                        nc.scalar.activation(ee[0:w, :], dd[0:w, :], AF.Exp,
                                             scale=-1.0)
                        ss = tails.tile([128, 1], F32, tag="ss")
                        nc.vector.tensor_reduce(ss[0:w, :], ee[0:w, :], AX.X,
                                                ALU.add)
                        rr = tails.tile([128, 1], F32, tag="rr")
                        nc.vector.reciprocal(rr[0:w, :], ss[0:w, :])
                        nc.vector.tensor_scalar(
                            score_col[0:w, it:it + 1], d3[0:w, 0:1],
                            rr[0:w, 0:1], None, ALU.mult,
                        )
            nc.sync.dma_start(score_d[:], score_col[:])

    nc.compile()
    return nc


def _get_program():
    if "nc" not in _cache:
        _cache["nc"] = _build_program()
    return _cache["nc"]


def kernel(p, W, b, C):
    from concourse.bass_utils import run_bass_kernel_spmd

    nc = _get_program()

    p = np.ascontiguousarray(np.asarray(p, dtype=np.float32))
    W = np.asarray(W, dtype=np.float32)
    b = np.ascontiguousarray(np.asarray(b, dtype=np.float32))
    C = np.ascontiguousarray(np.asarray(C, dtype=np.float32))

    wt = np.ascontiguousarray(W.T)                                # [c, d]
    cn = np.sum(C.astype(np.float64) * C, axis=0).astype(np.float32)
    cblock = np.broadcast_to((-cn / 128.0)[None, :], (128, P))
    cb = np.ascontiguousarray(
        np.concatenate([2.0 * C, cblock], axis=0)                 # [1920, P]
    )
    onec = np.ones((128, 1), dtype=np.float32)
    oner = np.ones((1, 128), dtype=np.float32)
    ones2 = np.ones((128, 128), dtype=np.float32)

    p_flat = p.reshape(B, DIM, HW)
    in_maps = []
    for core in range(NCORES):
        bidx, half = divmod(core, 2)
        pT = np.ascontiguousarray(p_flat[bidx, :, half * HALF:(half + 1) * HALF])
        in_maps.append({
            "pT": pT, "wt": wt, "cb": cb, "bias": b,
            "onec": onec, "oner": oner, "ones2": ones2,
        })

    _cache["last_in_maps"] = in_maps
    res = run_bass_kernel_spmd(nc, in_maps, list(range(NCORES)))
    _cache["last_result"] = res

    return assemble_output(per_core=[res.results[c]["score"] for c in range(NCORES)])


def assemble_output(per_core=None, res_concat=None):
    if per_core is None:
        sc_all = res_concat["score"]                              # [8*128, 13]
        per_core = [sc_all[c * 128:(c + 1) * 128] for c in range(NCORES)]
    out = np.empty((B, 1, H, W_), dtype=np.float32)
    for core in range(NCORES):
        bidx, half = divmod(core, 2)
        sc = per_core[core]                                       # [128, 13]
        flat = np.empty(HALF, dtype=np.float32)
        flat[:12 * 128] = sc[:, :12].T.reshape(-1)
        flat[12 * 128:] = sc[:LAST_W, 12]
        out.reshape(B, 1, HW)[bidx, 0, half * HALF:(half + 1) * HALF] = flat
    return out



# revision 7
# speedup vs baseline: 2.0266x; 2.0266x over previous
"""Trainium2 Bass kernel for nn_DSVDD (retrieval_knn) — fp8 DoubleRow.

Math (per batch b):
  phi = W @ p_b + bias            [DIM, HW]    (1x1 conv)
  sqdist[i,j] = ||phi_i||^2 + ||C_j||^2 - 2 phi_i . C_j
  top-3 smallest distances d0<=d1<=d2  ->  w0 = 1/(1+exp(d0-d1)+exp(d0-d2))
  score[i] = w0 * d0

Device strategy (8 cores, data-parallel over (batch, HW-half)):
  Both GEMMs run as fp8e4m3 DoubleRow matmuls (256-row contraction per
  instruction, 2 MACs/PE/cycle — measured 1.06 cyc/col on HW vs 2.2 for
  fp32r at 224 cols).  Host pre-scales W*1024, p*16, phi*16, 2C*512 (all
  pow-2, maxima <=133 vs fp8 max 240); PSUM stays fp32 so only operand
  quantization (~3% rel) enters, final rel err ~7e-3 vs 2e-2 gate.
  conv: phi_q (fp8, s1-scaled) via ACT; f=||phi||^2 via ACT Square from
  PSUM accumulated on GpSimd.  G: Y = s1*s2*2phi.C per 224-wide j-slice,
  -c*s1*s2 added on GpSimd, DVE max8 per slice into slots + one final
  max8 per i-tile (top-3 largest Y == top-3 smallest sqdist).  All cb
  slices SBUF-resident (loaded during conv).  Tail (sqrt, softmin) on
  ACT/DVE overlaps the next i-tile's matmuls.
"""
import sys

sys.path.insert(0, "/opt/trn_rl_repo")

import numpy as np

B, DIM, H, W_ = 4, 1792, 56, 56
HW = H * W_            # 3136
P = 3136               # prototypes
NCORES = 8
HALF = HW // 2         # 1568 positions per core
KC = DIM // 128        # 14 contraction chunks
NPAIR = KC // 2        # 7 DoubleRow pairs
IB = 224               # conv i-block (moving cols)
NIB = HALF // IB       # 7
JS = 224               # G j-slice width
NJS = P // JS          # 14
NIT = 13               # i-tiles: 12 full + 1 ragged(32)
LAST_W = HALF - 12 * 128   # 32
S_W, S_P, S1, S2 = 1024.0, 16.0, 16.0, 512.0
NWARM = 16

_cache = {}


def _build_program():
    import concourse.tile as tile
    from concourse import bacc, mybir

    F32 = mybir.dt.float32
    F32R = mybir.dt.float32r
    F8 = mybir.dt.float8e4
    AF = mybir.ActivationFunctionType
    ALU = mybir.AluOpType
    AX = mybir.AxisListType
    PM = mybir.MatmulPerfMode

    nc = bacc.Bacc("TRN2", target_bir_lowering=False, debug=False)

    # host-relaid layouts: big contiguous per-partition runs for DMA
    pt_d = nc.dram_tensor("pt", [NIB * 128, KC * IB], F8, kind="ExternalInput")
    wt_d = nc.dram_tensor("wt", [KC * 128, KC * 128], F8, kind="ExternalInput")
    cb_d = nc.dram_tensor("cb", [NJS * 128, KC * JS], F8, kind="ExternalInput")
    cbc_d = nc.dram_tensor("cbc", [NJS * 128, JS], F32, kind="ExternalInput")
    b1_d = nc.dram_tensor("b1", [DIM], F32, kind="ExternalInput")   # b * S1
    b0_d = nc.dram_tensor("b0", [DIM], F32, kind="ExternalInput")   # b
    onec_d = nc.dram_tensor("onec", [128, 1], F32, kind="ExternalInput")
    oner_d = nc.dram_tensor("oner", [1, 128], F32R, kind="ExternalInput")
    score_d = nc.dram_tensor("score", [128, NIT], F32, kind="ExternalOutput")

    with tile.TileContext(nc) as tc:
        with (
            tc.tile_pool(name="persist", bufs=1) as persist,
            tc.tile_pool(name="wtp", bufs=KC) as wtp,
            tc.tile_pool(name="ptp", bufs=NIB) as ptp,
            tc.tile_pool(name="cbp", bufs=NJS) as cbp,
            tc.tile_pool(name="ccp", bufs=NJS) as ccp,
        ):
            phi = persist.tile([128, KC, HALF], F8)
            b1c = persist.tile([128, KC], F32)
            b0c = persist.tile([128, KC], F32)
            onec = persist.tile([128, 1], F32)
            oner = persist.tile([1, 128], F32R)
            warm = persist.tile([128, 512], F32R)
            fsum = persist.tile([128, HALF], F32)
            f_row = persist.tile([1, HALF], F32)
            f_col = persist.tile([128, NIT], F32)
            runA = persist.tile([128, NIT, 8], F32)
            score_col = persist.tile([128, NIT], F32)

            def load_wt(dcg):
                t = wtp.tile([128, KC, 128], F8, name="wt_t")
                nc.sync.dma_start(
                    t[:],
                    wt_d[dcg * 128:(dcg + 1) * 128, :].rearrange(
                        "p (cc d) -> p cc d", cc=KC),
                )
                return t

            def load_pt(ib):
                t = ptp.tile([128, KC, IB], F8, name="pt_t")
                nc.sync.dma_start(
                    t[:],
                    pt_d[ib * 128:(ib + 1) * 128, :].rearrange(
                        "p (cc i) -> p cc i", cc=KC),
                )
                return t

            # priority DMA order: conv startup first, then the rest
            wt_t = {0: load_wt(0)}
            pt_t = [load_pt(ib) for ib in range(NIB)]
            nc.sync.dma_start(b1c[:], b1_d.rearrange("(g p) -> p g", p=128))
            nc.sync.dma_start(b0c[:], b0_d.rearrange("(g p) -> p g", p=128))
            nc.sync.dma_start(onec[:], onec_d[:])
            nc.sync.dma_start(oner[:], oner_d[:])
            for dcg in range(1, KC):
                wt_t[dcg] = load_wt(dcg)
            cb_t, cc_t = [], []
            for js in range(NJS):
                t = cbp.tile([128, KC, JS], F8, name="cb_t")
                nc.sync.dma_start(
                    t[:],
                    cb_d[js * 128:(js + 1) * 128, :].rearrange(
                        "p (cc j) -> p cc j", cc=KC),
                )
                cb_t.append(t)
                t2 = ccp.tile([128, JS], F32, name="cc_t")
                nc.sync.dma_start(t2[:], cbc_d[js * 128:(js + 1) * 128, :])
                cc_t.append(t2)

            # ------------- conv phase: phi = W @ p + b, f = ||phi||^2 -------
            with (
                tc.tile_pool(name="sqp", bufs=4) as sqp,
                tc.tile_pool(name="cps", bufs=4, space="PSUM") as cps,
                tc.tile_pool(name="wps", bufs=1, space="PSUM") as wps,
            ):
                # PE warmup: ramp the clock to 2.4 GHz while DMAs land
                nc.vector.memset(warm[:].bitcast(F32), 1.0)
                wacc = wps.tile([128, 512], F32, name="wacc", tag="w")
                for _ in range(NWARM):
                    nc.tensor.matmul(wacc[:], warm[:, 0:128], warm[:],
                                     start=True, stop=True)

                for dcg in range(KC):
                    for ib in range(NIB):
                        acc = cps.tile([128, IB], F32, name="acc", tag="acc")
                        for pr in range(NPAIR):
                            nc.tensor.matmul(
                                acc[:],
                                wt_t[dcg][:, 2 * pr:2 * pr + 2, :],
                                pt_t[ib][:, 2 * pr:2 * pr + 2, :],
                                start=(pr == 0),
                                stop=(pr == NPAIR - 1),
                                perf_mode=PM.DoubleRow,
                            )
                        isl = slice(ib * IB, (ib + 1) * IB)
                        # phi_q = (psum/(s_w*s_p) + b) * s1, rounded to fp8
                        nc.scalar.activation(
                            phi[:, dcg, isl], acc[:], AF.Identity,
                            bias=b1c[:, dcg:dcg + 1], scale=S1 / (S_W * S_P),
                        )
                        # sq = (psum/(s_w*s_p) + b)^2 — exact-phi square
                        sq = sqp.tile([128, IB], F32, name="sq", tag="sq")
                        nc.scalar.activation(
                            sq[:], acc[:], AF.Square,
                            bias=b0c[:, dcg:dcg + 1], scale=1.0 / (S_W * S_P),
                        )
                        if dcg == 0:
                            nc.gpsimd.tensor_copy(fsum[:, isl], sq[:])
                        else:
                            nc.gpsimd.tensor_tensor(
                                fsum[:, isl], fsum[:, isl], sq[:], ALU.add)

            # ------------- f reduce + relayout: [128,1568] -> [128,13] ------
            with (
                tc.tile_pool(name="fps", bufs=4, space="PSUM") as fps,
                tc.tile_pool(name="ftp", bufs=1, space="PSUM") as ftp,
            ):
                for k, (off, wid) in enumerate(
                        ((0, 512), (512, 512), (1024, 512), (1536, 32))):
                    fp = fps.tile([1, 512], F32, name="fp", tag="f")
                    nc.tensor.matmul(fp[0:1, 0:wid], onec[:],
                                     fsum[:, off:off + wid],
                                     start=True, stop=True)
                    nc.scalar.activation(f_row[0:1, off:off + wid],
                                         fp[0:1, 0:wid], AF.Copy)
                ft = ftp.tile([128, NIT], F32, name="ft", tag="ft")
                for it in range(NIT):
                    w = 128 if it < 12 else LAST_W
                    nc.tensor.transpose(
                        ft[0:w, it:it + 1],
                        f_row[:, it * 128:it * 128 + w],
                        oner[0:1, 0:1].bitcast(F32),
                    )
                nc.scalar.activation(f_col[:], ft[:], AF.Copy)

            # ------------- G phase: Y = s1*s2*(2 phi.C - c), top-8 ----------
            with (
                tc.tile_pool(name="ysb", bufs=6) as ysb,
                tc.tile_pool(name="m8p", bufs=2) as m8p,
                tc.tile_pool(name="tails", bufs=4) as tails,
                tc.tile_pool(name="yps", bufs=7, space="PSUM") as yps,
            ):
                for it in range(NIT):
                    w = 128 if it < 12 else LAST_W
                    i0 = it * 128
                    m8 = m8p.tile([128, NJS, 8], F32, name="m8", tag="m8")
                    for js in range(NJS):
                        y = yps.tile([128, JS], F32, name="y", tag="y")
                        for pr in range(NPAIR):
                            nc.tensor.matmul(
                                y[0:w, :],
                                phi[:, 2 * pr:2 * pr + 2, i0:i0 + w],
                                cb_t[js][:, 2 * pr:2 * pr + 2, :],
                                start=(pr == 0),
                                stop=(pr == NPAIR - 1),
                                perf_mode=PM.DoubleRow,
                            )
                        ys = ysb.tile([128, JS], F32, name="ys", tag="ys")
                        nc.vector.tensor_tensor(
                            ys[0:w, :], y[0:w, :], cc_t[js][0:w, :], ALU.add)
                        nc.vector.max(m8[0:w, js, :], ys[0:w, :])
                    nc.vector.max(runA[0:w, it, :], m8[0:w, :, :])

                    # tail: d = sqrt(f - Y/(s1*s2)), softmin weight
                    d3 = tails.tile([128, 3], F32, tag="d3")
                    nc.scalar.activation(
                        d3[0:w, :], runA[0:w, it, 0:3], AF.Sqrt,
                        bias=f_col[0:w, it:it + 1], scale=-1.0 / (S1 * S2),
                    )
                    dd = tails.tile([128, 3], F32, tag="dd")
                    nc.vector.tensor_scalar(
                        dd[0:w, :], d3[0:w, :], d3[0:w, 0:1], None,
                        ALU.subtract,
                    )
                    ee = tails.tile([128, 3], F32, tag="ee")
                    nc.scalar.activation(ee[0:w, :], dd[0:w, :], AF.Exp,
                                         scale=-1.0)
                    ss = tails.tile([128, 1], F32, tag="ss")
                    nc.vector.tensor_reduce(ss[0:w, :], ee[0:w, :], AX.X,
                                            ALU.add)
                    rr = tails.tile([128, 1], F32, tag="rr")
                    nc.vector.reciprocal(rr[0:w, :], ss[0:w, :])
                    nc.vector.tensor_scalar(
                        score_col[0:w, it:it + 1], d3[0:w, 0:1],
                        rr[0:w, 0:1], None, ALU.mult,
                    )
            nc.sync.dma_start(score_d[:], score_col[:])

    nc.compile()
    return nc


def _get_program():
    if "nc" not in _cache:
        _cache["nc"] = _build_program()
    return _cache["nc"]


def _q8(x, s):
    import ml_dtypes
    y = np.asarray(x * np.float32(s), dtype=ml_dtypes.float8_e4m3)
    return y


def kernel(p, W, b, C):
    from concourse.bass_utils import run_bass_kernel_spmd

    nc = _get_program()

    p = np.ascontiguousarray(np.asarray(p, dtype=np.float32))
    W = np.asarray(W, dtype=np.float32)
    b = np.ascontiguousarray(np.asarray(b, dtype=np.float32))
    C = np.ascontiguousarray(np.asarray(C, dtype=np.float32))

    # dcg-major W^T: wt[dcg*128+p, cc*128+dd] = W[dcg*128+dd, cc*128+p]*S_W
    Wq = _q8(W, S_W).reshape(KC, 128, KC, 128)            # [dcg, dd, cc, p]
    wt = np.ascontiguousarray(
        Wq.transpose(0, 3, 2, 1).reshape(DIM, DIM))       # [(dcg p), (cc dd)]

    # js-major prototype bank: cb[js*128+p, cc*224+jj] = 2C[cc*128+p, js*224+jj]*S2
    Cq = _q8(2.0 * C, S2).reshape(KC, 128, NJS, JS)       # [cc, p, js, jj]
    cb = np.ascontiguousarray(
        Cq.transpose(2, 1, 0, 3).reshape(NJS * 128, KC * JS))

    cn = np.sum(C.astype(np.float64) * C, axis=0).astype(np.float32)  # [P]
    cbc = np.ascontiguousarray(np.broadcast_to(
        (-cn * np.float32(S1 * S2)).reshape(NJS, 1, JS),
        (NJS, 128, JS)).reshape(NJS * 128, JS))

    b1 = np.ascontiguousarray(b * np.float32(S1))
    onec = np.ones((128, 1), dtype=np.float32)
    oner = np.ones((1, 128), dtype=np.float32)

    # ib-major p shards: pt[ib*128+p, cc*224+ii] = p[cc*128+p, ib*224+ii]*S_P
    p_flat = p.reshape(B, DIM, HW)
    in_maps = []
    for core in range(NCORES):
        bidx, half = divmod(core, 2)
        pq = _q8(p_flat[bidx, :, half * HALF:(half + 1) * HALF], S_P)
        pt = np.ascontiguousarray(
            pq.reshape(KC, 128, NIB, IB).transpose(2, 1, 0, 3).reshape(
                NIB * 128, KC * IB))
        in_maps.append({
            "pt": pt, "wt": wt, "cb": cb, "cbc": cbc,
            "b1": b1, "b0": b, "onec": onec, "oner": oner,
        })

    _cache["last_in_maps"] = in_maps
    res = run_bass_kernel_spmd(nc, in_maps, list(range(NCORES)))
    _cache["last_result"] = res

    return assemble_output(per_core=[res.results[c]["score"] for c in range(NCORES)])


def assemble_output(per_core=None, res_concat=None):
    if per_core is None:
        sc_all = res_concat["score"]                              # [8*128, 13]
        per_core = [sc_all[c * 128:(c + 1) * 128] for c in range(NCORES)]
    out = np.empty((B, 1, H, W_), dtype=np.float32)
    for core in range(NCORES):
        bidx, half = divmod(core, 2)
        sc = per_core[core]                                       # [128, 13]
        flat = np.empty(HALF, dtype=np.float32)
        flat[:12 * 128] = sc[:, :12].T.reshape(-1)
        flat[12 * 128:] = sc[:LAST_W, 12]
        out.reshape(B, 1, HW)[bidx, 0, half * HALF:(half + 1) * HALF] = flat
    return out
